# revision 1
# baseline (speedup 1.0000x reference)
"""GQA attention kernel for Trainium2, 8-core SPMD.

Sharding: tensor-parallel=4 over kv-head pairs x data-parallel=2 over batch.
Each core: one batch, 8 q-heads, 2 kv-heads, full 2048-token sequence.
All matmuls bf16 inputs / fp32 PSUM accumulation.

v2 design notes (informed by v1 trace):
  - softmax denominator: e_sum accumulated on DVE only (gpsimd/Pool ops have
    ~1.2us launch overhead and paced the whole phase 2 in v1), then one
    ones-matmul per head per q-chunk
  - head-PAIR attention: scores for 2 heads of the same kv-head go into one
    [128,2,512] PSUM tile (2 banks, bank-aligned halves) so exp runs as a
    single [128,1024] ACT op (halves ACT instruction count + bubbles)
  - reciprocal -> reciprocal_approx_fast (plain InstReciprocal measured 3.3us)
  - V projection via v^T (long 512-streams) + PE transposes, instead of 512
    short FD=256 matmuls
  - O-projection of chunk qc-1 interleaved between head-pairs of chunk qc to
    fill PE under the ACT-paced attention (keeps HAM warm, no big PE gaps)
  - wq/wk host layout [h,p,c,m] so weight-slab DMAs are 8KB-contiguous
    (256B-elem strided DMAs pay a 2x latency penalty)
"""
import numpy as np
import ml_dtypes

import concourse.bacc as bacc
import concourse.bass as bass
import concourse.tile as tile
from concourse import mybir
from concourse.bass_utils import run_bass_kernel_spmd

BF = mybir.dt.bfloat16
F32 = mybir.dt.float32
BF_NP = np.dtype(ml_dtypes.bfloat16)

B, S, HIDDEN = 2, 2048, 4096
NUM_HEADS, NUM_KV_HEADS, HEAD_DIM = 32, 8, 128
GROUPS = NUM_HEADS // NUM_KV_HEADS
ROPE_THETA = 10000.0
TP = 4

FULL_CFG = dict(S=2048, HID=4096, NQ=8, NKV=2, SB=512, QC=512)


def build_nc(cfg):
    S_, HID, NQ, NKV, SB, QC = (cfg[k] for k in ("S", "HID", "NQ", "NKV", "SB", "QC"))
    HC = HID // 128
    NB = S_ // SB
    NQC = S_ // QC
    KT = S_ // 128
    DV = NKV * 128
    NO = HID // 512
    NHP = NQ // 2            # head pairs
    scale = 1.0 / np.sqrt(128.0)

    nc = bacc.Bacc("TRN2", target_bir_lowering=False, debug=False)
    xt = nc.dram_tensor("xt", (HC, 128, S_), BF, kind="ExternalInput").ap()
    wq = nc.dram_tensor("wq", (NQ, 128, HC, 128), BF, kind="ExternalInput").ap()
    wk = nc.dram_tensor("wk", (NKV, 128, HC, 128), BF, kind="ExternalInput").ap()
    wv = nc.dram_tensor("wv", (HC, 128, DV), BF, kind="ExternalInput").ap()
    wo = nc.dram_tensor("wo", (NO, 128, NQ, 512), BF, kind="ExternalInput").ap()
    cosd = nc.dram_tensor("cos", (128, S_), BF, kind="ExternalInput").ap()
    sind = nc.dram_tensor("sin", (128, S_), BF, kind="ExternalInput").ap()
    rmatd = nc.dram_tensor("rmat", (128, 128), BF, kind="ExternalInput").ap()
    identd = nc.dram_tensor("ident", (128, 128), BF, kind="ExternalInput").ap()
    o = nc.dram_tensor("o", (S_, HID), BF, kind="ExternalOutput").ap()

    with tile.TileContext(nc) as tc:
        with tc.tile_pool(name="cons", bufs=1) as cons, \
             tc.tile_pool(name="big", bufs=1) as big:
            r_sb = cons.tile([128, 128], BF, name="r_sb")
            id_sb = cons.tile([128, 128], BF, name="id_sb")
            ones_sb = cons.tile([128, 128], BF, name="ones_sb")
            nc.sync.dma_start(out=r_sb, in_=rmatd)
            nc.sync.dma_start(out=id_sb, in_=identd)
            nc.vector.memset(ones_sb, 1.0)

            q_sb = big.tile([128, NQ, S_], BF, name="q_sb")
            k_sb = big.tile([128, NKV, S_], BF, name="k_sb")
            v_sb = big.tile([128, KT, DV], BF, name="v_sb")

            # ---------------- phase 1: projections + rope ----------------
            with tc.tile_pool(name="p1c", bufs=1) as p1c, \
                 tc.tile_pool(name="xp", bufs=2) as xp, \
                 tc.tile_pool(name="wp", bufs=4) as wp, \
                 tc.tile_pool(name="rt", bufs=4) as rt, \
                 tc.tile_pool(name="vt", bufs=2) as vt, \
                 tc.tile_pool(name="pp", bufs=2, space="PSUM") as pp, \
                 tc.tile_pool(name="rp", bufs=2, space="PSUM") as rp, \
                 tc.tile_pool(name="vtp", bufs=2, space="PSUM") as vtp, \
                 tc.tile_pool(name="tpp", bufs=2, space="PSUM") as tpp:
                cos_sb = p1c.tile([128, S_], BF, name="cos_sb")
                sin_sb = p1c.tile([128, S_], BF, name="sin_sb")
                wv_sb = p1c.tile([128, HC, DV], BF, name="wv_sb")
                # first K slabs lead the queues so PE starts within ~5us
                wk_first = [wp.tile([128, HC, 128], BF, name="w_slab")
                            for _ in range(NKV)]
                for h in range(NKV):
                    for c4 in range(0, HC, 4):
                        nc.sync.dma_start(out=wk_first[h][:, c4:c4 + 4, :],
                                          in_=wk[h][:, c4:c4 + 4, :])
                nc.sync.dma_start(out=cos_sb, in_=cosd)
                nc.sync.dma_start(out=sin_sb, in_=sind)

                # delayed emission of PE ops whose inputs come off ACT/DVE,
                # so the PE queue never waits on a slow-engine producer
                pending = []

                def flush_pending():
                    while pending:
                        pending.pop(0)()

                xt_r = xt.rearrange("c p s -> p c s")
                wv_r = wv.rearrange("c p v -> p c v")
                for sb_i in range(NB):
                    ssl = slice(sb_i * SB, (sb_i + 1) * SB)
                    xt_t = xp.tile([128, HC, SB], BF, name="xt_t")
                    for c in range(HC):
                        nc.sync.dma_start(out=xt_t[:, c, :], in_=xt_r[:, c, ssl])
                    if sb_i == 0:
                        for c in range(HC):
                            nc.sync.dma_start(out=wv_sb[:, c, :], in_=wv_r[:, c, :])

                    # weight-slab prefetch: slab h+1's DMA issues while h runs
                    slabq = []

                    def slab_for(wten, h, sb_i=sb_i):
                        if sb_i == 0 and wten is wk:
                            return wk_first[h]
                        t = wp.tile([128, HC, 128], BF, name="w_slab")
                        nc.sync.dma_start(out=t, in_=wten[h])
                        return t

                    # K projection + rope, then v^T + transpose, then Q + rope
                    plan = [("k", NKV, wk, k_sb), ("v", NKV, wv, None),
                            ("q", NQ, wq, q_sb)]
                    slab_src = [(wten, h) for which, nheads, wten, _ in plan
                                if which != "v" for h in range(nheads)]
                    slabq = [slab_for(*slab_src[0]), slab_for(*slab_src[1])]
                    slab_i = 0
                    for which, nheads, wten, dst in plan:
                        for h in range(nheads):
                            if which == "v":
                                # v^T[d, tok] via long streams, then transpose
                                vt_ps = vtp.tile([128, SB], F32, name="vt_ps")
                                for c in range(HC):
                                    nc.tensor.matmul(
                                        vt_ps, wv_sb[:, c, h * 128:(h + 1) * 128],
                                        xt_t[:, c, :],
                                        start=(c == 0), stop=(c == HC - 1))
                                vt_sb = vt.tile([128, SB], BF, name="vt_sb")
                                nc.scalar.activation(
                                    out=vt_sb, in_=vt_ps,
                                    func=mybir.ActivationFunctionType.Copy)

                                def mk_trans(vt_sb=vt_sb, sb_i=sb_i, h=h):
                                    for t in range(SB // 128):
                                        tp_ps = tpp.tile([128, 128], BF,
                                                         name="tp_ps")
                                        nc.tensor.transpose(
                                            tp_ps,
                                            vt_sb[:, t * 128:(t + 1) * 128],
                                            id_sb)
                                        kt_i = sb_i * (SB // 128) + t
                                        nc.vector.tensor_copy(
                                            v_sb[:, kt_i, h * 128:(h + 1) * 128],
                                            tp_ps)
                                pending.append(mk_trans)
                                if len(pending) >= 2:
                                    pending.pop(0)()
                                continue
                            ps = pp.tile([128, SB], F32, name="ps_proj")
                            wslab = slabq[slab_i]
                            slab_i += 1
                            if slab_i + 1 < len(slab_src):
                                slabq.append(slab_for(*slab_src[slab_i + 1]))
                            for c in range(HC):
                                nc.tensor.matmul(ps, wslab[:, c, :], xt_t[:, c, :],
                                                 start=(c == 0), stop=(c == HC - 1))
                            # rope: dst = ps*cos + (R@ps)*sin_signed
                            qbf = rt.tile([128, SB], BF, name="rope_bf")
                            nc.scalar.activation(
                                out=qbf, in_=ps,
                                func=mybir.ActivationFunctionType.Copy)

                            def mk_rope(ps=ps, qbf=qbf, dst=dst, h=h, ssl=ssl):
                                rot = rp.tile([128, SB], F32, name="rot_ps")
                                nc.tensor.matmul(rot, r_sb, qbf,
                                                 start=True, stop=True)
                                t1 = rt.tile([128, SB], BF, name="rope_t1")
                                t2 = rt.tile([128, SB], BF, name="rope_t2")
                                nc.vector.tensor_mul(t1, ps, cos_sb[:, ssl])
                                nc.vector.tensor_mul(t2, rot, sin_sb[:, ssl])
                                nc.vector.tensor_add(dst[:, h, ssl], t1, t2)
                            pending.append(mk_rope)
                            if len(pending) >= 2:
                                pending.pop(0)()
                flush_pending()

            # ------------- phase 2+3: attention + output projection -------------
            with tc.tile_pool(name="wob", bufs=1) as wob, \
                 tc.tile_pool(name="aq", bufs=2) as aq, \
                 tc.tile_pool(name="ep", bufs=6) as ep, \
                 tc.tile_pool(name="es", bufs=3) as es, \
                 tc.tile_pool(name="rb", bufs=2) as rb, \
                 tc.tile_pool(name="ob", bufs=6) as ob, \
                 tc.tile_pool(name="sp", bufs=2, space="PSUM") as sp, \
                 tc.tile_pool(name="ap_", bufs=1, space="PSUM") as ap_, \
                 tc.tile_pool(name="op", bufs=1, space="PSUM") as op:
                wo_sb = wob.tile([128, NO, NQ, 512], BF, name="wo_sb")
                for n in range(NO):
                    nc.sync.dma_start(out=wo_sb[:, n], in_=wo[n])

                at_tiles = {}
                # O-projection emitted as fine-grained 4-matmul chunks injected
                # between attention k-tiles, so PE never idles in bursts (and
                # HAM never re-throttles). Single-n groups of 8 accumulating
                # matmuls, split in half, alternating between 2 PSUM tiles.
                oproj_chunks = []

                def emit_oproj_chunk():
                    qc, tt, n, half, g = oproj_chunks.pop(0)
                    at_qc = at_tiles[qc]
                    tsl = slice(tt * 128, (tt + 1) * 128)
                    name = "ps_o" + "ab"[g % 2]
                    if half == 0:
                        t = op.tile([128, 512], F32, name=name)
                        op_live[g % 2] = t
                        for c in range(4):
                            nc.tensor.matmul(
                                t, at_qc[:, c, tsl], wo_sb[:, n, c, :],
                                start=(c == 0), stop=False,
                                skip_group_check=True)
                    else:
                        t = op_live[g % 2]
                        for c in range(4, NQ):
                            nc.tensor.matmul(
                                t, at_qc[:, c, tsl], wo_sb[:, n, c, :],
                                start=False, stop=(c == NQ - 1),
                                skip_group_check=True)
                        o_t = ob.tile([128, 512], BF, name="o_t")
                        if n % 2 == 0:
                            nc.vector.tensor_copy(o_t, t)
                        else:
                            nc.scalar.activation(
                                out=o_t, in_=t,
                                func=mybir.ActivationFunctionType.Copy)
                        nc.sync.dma_start(
                            out=o[qc * QC + tt * 128:qc * QC + (tt + 1) * 128,
                                  n * 512:(n + 1) * 512],
                            in_=o_t)

                op_live = [None, None]

                def queue_oproj(qc):
                    for g in range(32):
                        n = g // 4          # n-major: early groups need only
                        tt = g % 4          # the first wo chunks
                        oproj_chunks.append((qc, tt, n, 0, g))
                        oproj_chunks.append((qc, tt, n, 1, g))

                norm_pending = []
                for qc in range(NQC):
                    qsl = slice(qc * QC, (qc + 1) * QC)
                    at_qc = aq.tile([128, NQ, QC], BF, name="at_qc")
                    at_tiles[qc] = at_qc
                    for hp in range(NHP):
                        if norm_pending:
                            norm_pending.pop(0)()
                        h0 = 2 * hp
                        kvh = h0 // (NQ // NKV)
                        s2 = sp.tile([128, 2, QC], F32, name="s2")
                        attn2 = ap_.tile([128, 2, QC], F32, name="attn2")
                        esd = es.tile([128, 2, QC], BF, name="esd")
                        for kc in range(KT):
                            ksl = k_sb[:, kvh, kc * 128:(kc + 1) * 128]
                            nc.tensor.matmul(s2[:, 0, :], ksl, q_sb[:, h0, qsl],
                                             start=True, stop=True,
                                             skip_group_check=True)
                            nc.tensor.matmul(s2[:, 1, :], ksl, q_sb[:, h0 + 1, qsl],
                                             start=True, stop=True,
                                             skip_group_check=True)
                            # exp of k-tile 0 goes straight into esd (saves a copy)
                            e2 = esd if kc == 0 else ep.tile([128, 2, QC], BF,
                                                             name="e2")
                            nc.scalar.activation(
                                out=e2, in_=s2,
                                func=mybir.ActivationFunctionType.Exp,
                                scale=scale)
                            vsl = v_sb[:, kc, kvh * 128:(kvh + 1) * 128]
                            nc.tensor.matmul(attn2[:, 0, :], vsl, e2[:, 0, :],
                                             start=(kc == 0), stop=(kc == KT - 1),
                                             skip_group_check=True)
                            nc.tensor.matmul(attn2[:, 1, :], vsl, e2[:, 1, :],
                                             start=(kc == 0), stop=(kc == KT - 1),
                                             skip_group_check=True)
                            if kc > 0:
                                nc.vector.tensor_add(esd, esd, e2)
                            if oproj_chunks:
                                emit_oproj_chunk()
                        # evict raw attention on ACT (frees PSUM fast); the
                        # normalize chain is all-DVE and off the hot path
                        at_un = es.tile([128, 2, QC], BF, name="at_un")
                        nc.scalar.activation(
                            out=at_un, in_=attn2,
                            func=mybir.ActivationFunctionType.Copy)
                        # den via all-ones stationary: every output partition
                        # gets the full k-sum, so the reciprocal is already
                        # broadcast. Rides the s2 slot ring to stay in 8 banks.
                        den_bc = sp.tile([128, 2, QC], F32, name="s2")
                        nc.tensor.matmul(den_bc[:, 0, :], ones_sb, esd[:, 0, :],
                                         start=True, stop=True,
                                         skip_group_check=True)
                        nc.tensor.matmul(den_bc[:, 1, :], ones_sb, esd[:, 1, :],
                                         start=True, stop=True,
                                         skip_group_check=True)
                        rec2 = rb.tile([128, 2, QC], F32, name="rec2")
                        nc.vector.reciprocal_approx_fast(out=rec2, in_=den_bc)
                        norm_pending.append(
                            lambda at_qc=at_qc, h0=h0, at_un=at_un, rec2=rec2:
                            nc.vector.tensor_mul(at_qc[:, h0:h0 + 2, :], at_un,
                                                 rec2))
                    queue_oproj(qc)
                while norm_pending:
                    norm_pending.pop(0)()
                while oproj_chunks:
                    emit_oproj_chunk()
    nc.compile()
    return nc


def _rope_tables(position_ids_b, S_):
    """cos/sin tables in [d=128, s] layout, sin sign-folded for the half-swap."""
    pos = position_ids_b.astype(np.float32)
    inv_freq = (1.0 / (ROPE_THETA ** (np.arange(0, HEAD_DIM, 2, dtype=np.float32)
                                      / HEAD_DIM))).astype(np.float32)
    freqs = pos[:, None] * inv_freq[None, :]          # [s, 64]
    emb = np.concatenate([freqs, freqs], axis=1)      # [s, 128]
    cos = np.cos(emb).T.copy()                        # [128, s]
    sin = np.sin(emb).T.copy()
    sin[:64] *= -1.0                                  # sign-fold for swap rope
    return cos.astype(BF_NP), sin.astype(BF_NP)


def _prep_core_inputs(hidden_states, position_ids, Wq, Wk, Wv, Wo):
    rmat = np.zeros((128, 128), dtype=np.float32)
    for i in range(128):
        rmat[i, (i + 64) % 128] = 1.0
    rmat = rmat.astype(BF_NP)
    ident = np.eye(128, dtype=np.float32).astype(BF_NP)

    HC = HIDDEN // 128
    in_maps = []
    for t in range(TP):
        fq = slice(1024 * t, 1024 * (t + 1))
        fkv = slice(256 * t, 256 * (t + 1))
        # [h, p, c, m] layout: slab DMA reads 8KB contiguous per partition
        wq_t = np.ascontiguousarray(
            Wq[:, fq].reshape(HC, 128, 8, 128).transpose(2, 1, 0, 3)).astype(BF_NP)
        wk_t = np.ascontiguousarray(
            Wk[:, fkv].reshape(HC, 128, 2, 128).transpose(2, 1, 0, 3)).astype(BF_NP)
        wv_t = np.ascontiguousarray(Wv[:, fkv].reshape(HC, 128, 256)).astype(BF_NP)
        wo_t = np.ascontiguousarray(
            Wo[fq, :].reshape(8, 128, 8, 512).transpose(2, 1, 0, 3)).astype(BF_NP)
        for b in range(B):
            xt = np.ascontiguousarray(
                hidden_states[b].T.reshape(HC, 128, S)).astype(BF_NP)
            cos, sin = _rope_tables(position_ids[b], S)
            in_maps.append({"xt": xt, "wq": wq_t, "wk": wk_t, "wv": wv_t,
                            "wo": wo_t, "cos": cos, "sin": sin, "rmat": rmat,
                            "ident": ident})
    return in_maps


_NC_CACHE = {}


def kernel(hidden_states, position_ids, Wq, Wk, Wv, Wo):
    if "nc" not in _NC_CACHE:
        _NC_CACHE["nc"] = build_nc(FULL_CFG)
    nc = _NC_CACHE["nc"]
    in_maps = _prep_core_inputs(np.asarray(hidden_states), np.asarray(position_ids),
                                np.asarray(Wq), np.asarray(Wk),
                                np.asarray(Wv), np.asarray(Wo))
    res = run_bass_kernel_spmd(nc, in_maps, core_ids=list(range(8)))
    out = np.zeros((B, S, HIDDEN), dtype=np.float32)
    for t in range(TP):
        for b in range(B):
            out[b] += res.results[t * B + b]["o"].astype(np.float32)
    return out



# revision 3
# speedup vs baseline: 1.2874x; 1.2874x over previous
"""GQA attention kernel for Trainium2, 8-core SPMD.

Sharding: tensor-parallel=4 over kv-head pairs x data-parallel=2 over batch.
Each core: one batch, 8 q-heads, 2 kv-heads, full 2048-token sequence.
All matmuls bf16 inputs / fp32 PSUM accumulation.

v2 design notes (informed by v1 trace):
  - softmax denominator: e_sum accumulated on DVE only (gpsimd/Pool ops have
    ~1.2us launch overhead and paced the whole phase 2 in v1), then one
    ones-matmul per head per q-chunk
  - head-PAIR attention: scores for 2 heads of the same kv-head go into one
    [128,2,512] PSUM tile (2 banks, bank-aligned halves) so exp runs as a
    single [128,1024] ACT op (halves ACT instruction count + bubbles)
  - reciprocal -> reciprocal_approx_fast (plain InstReciprocal measured 3.3us)
  - V projection via v^T (long 512-streams) + PE transposes, instead of 512
    short FD=256 matmuls
  - O-projection of chunk qc-1 interleaved between head-pairs of chunk qc to
    fill PE under the ACT-paced attention (keeps HAM warm, no big PE gaps)
  - wq/wk host layout [h,p,c,m] so weight-slab DMAs are 8KB-contiguous
    (256B-elem strided DMAs pay a 2x latency penalty)
"""
import numpy as np
import ml_dtypes

import concourse.bacc as bacc
import concourse.bass as bass
import concourse.tile as tile
from concourse import mybir
from concourse.bass_utils import run_bass_kernel_spmd

BF = mybir.dt.bfloat16
F32 = mybir.dt.float32
BF_NP = np.dtype(ml_dtypes.bfloat16)

B, S, HIDDEN = 2, 2048, 4096
NUM_HEADS, NUM_KV_HEADS, HEAD_DIM = 32, 8, 128
GROUPS = NUM_HEADS // NUM_KV_HEADS
ROPE_THETA = 10000.0
TP = 4

FULL_CFG = dict(S=2048, HID=4096, NQ=8, NKV=2, SB=512, QC=512)


def build_nc(cfg):
    S_, HID, NQ, NKV, SB, QC = (cfg[k] for k in ("S", "HID", "NQ", "NKV", "SB", "QC"))
    HC = HID // 128
    NB = S_ // SB
    NQC = S_ // QC
    KT = S_ // 128
    DV = NKV * 128
    NO = HID // 512
    NHP = NQ // 2            # head pairs
    scale = 1.0 / np.sqrt(128.0)

    nc = bacc.Bacc("TRN2", target_bir_lowering=False, debug=False)
    xt = nc.dram_tensor("xt", (HC, 128, S_), BF, kind="ExternalInput").ap()
    wq = nc.dram_tensor("wq", (NQ, 128, HC, 128), BF, kind="ExternalInput").ap()
    wk = nc.dram_tensor("wk", (NKV, 128, HC, 128), BF, kind="ExternalInput").ap()
    wv = nc.dram_tensor("wv", (HC, 128, DV), BF, kind="ExternalInput").ap()
    wo = nc.dram_tensor("wo", (NO, 128, NQ, 512), BF, kind="ExternalInput").ap()
    cosd = nc.dram_tensor("cos", (128, S_), BF, kind="ExternalInput").ap()
    sind = nc.dram_tensor("sin", (128, S_), BF, kind="ExternalInput").ap()
    rmatd = nc.dram_tensor("rmat", (128, 128), BF, kind="ExternalInput").ap()
    identd = nc.dram_tensor("ident", (128, 128), BF, kind="ExternalInput").ap()
    o = nc.dram_tensor("o", (S_, HID), BF, kind="ExternalOutput").ap()

    with tile.TileContext(nc) as tc:
        with tc.tile_pool(name="cons", bufs=1) as cons, \
             tc.tile_pool(name="big", bufs=1) as big:
            r_sb = cons.tile([128, 128], BF, name="r_sb")
            id_sb = cons.tile([128, 128], BF, name="id_sb")
            ones_sb = cons.tile([128, 128], BF, name="ones_sb")
            nc.sync.dma_start(out=r_sb, in_=rmatd)
            nc.sync.dma_start(out=id_sb, in_=identd)
            nc.vector.memset(ones_sb, 1.0)

            q_sb = big.tile([128, NQ, S_], BF, name="q_sb")
            k_sb = big.tile([128, NKV, S_], BF, name="k_sb")
            v_sb = big.tile([128, KT, DV], BF, name="v_sb")

            # ---------------- phase 1: projections + rope ----------------
            with tc.tile_pool(name="p1c", bufs=1) as p1c, \
                 tc.tile_pool(name="xp", bufs=2) as xp, \
                 tc.tile_pool(name="wp", bufs=4) as wp, \
                 tc.tile_pool(name="rt", bufs=4) as rt, \
                 tc.tile_pool(name="vt", bufs=2) as vt, \
                 tc.tile_pool(name="pp", bufs=2, space="PSUM") as pp, \
                 tc.tile_pool(name="rp", bufs=2, space="PSUM") as rp, \
                 tc.tile_pool(name="vtp", bufs=2, space="PSUM") as vtp, \
                 tc.tile_pool(name="tpp", bufs=2, space="PSUM") as tpp:
                cos_sb = p1c.tile([128, S_], BF, name="cos_sb")
                sin_sb = p1c.tile([128, S_], BF, name="sin_sb")
                wv_sb = p1c.tile([128, HC, DV], BF, name="wv_sb")
                # first K slabs lead the queues so PE starts within ~5us
                wk_first = [wp.tile([128, HC, 128], BF, name="w_slab")
                            for _ in range(NKV)]
                for h in range(NKV):
                    for c4 in range(0, HC, 4):
                        nc.sync.dma_start(out=wk_first[h][:, c4:c4 + 4, :],
                                          in_=wk[h][:, c4:c4 + 4, :])
                nc.sync.dma_start(out=cos_sb, in_=cosd)
                nc.sync.dma_start(out=sin_sb, in_=sind)

                # delayed emission of PE ops whose inputs come off ACT/DVE,
                # so the PE queue never waits on a slow-engine producer
                pending = []

                def flush_pending():
                    while pending:
                        pending.pop(0)()

                xt_r = xt.rearrange("c p s -> p c s")
                wv_r = wv.rearrange("c p v -> p c v")
                for sb_i in range(NB):
                    ssl = slice(sb_i * SB, (sb_i + 1) * SB)
                    xt_t = xp.tile([128, HC, SB], BF, name="xt_t")
                    for c in range(HC):
                        nc.sync.dma_start(out=xt_t[:, c, :], in_=xt_r[:, c, ssl])
                    if sb_i == 0:
                        for c in range(HC):
                            nc.sync.dma_start(out=wv_sb[:, c, :], in_=wv_r[:, c, :])

                    # weight-slab prefetch: slab h+1's DMA issues while h runs
                    slabq = []

                    def slab_for(wten, h, sb_i=sb_i):
                        if sb_i == 0 and wten is wk:
                            return wk_first[h]
                        t = wp.tile([128, HC, 128], BF, name="w_slab")
                        nc.sync.dma_start(out=t, in_=wten[h])
                        return t

                    # K projection + rope, then v^T + transpose, then Q + rope
                    plan = [("k", NKV, wk, k_sb), ("v", NKV, wv, None),
                            ("q", NQ, wq, q_sb)]
                    slab_src = [(wten, h) for which, nheads, wten, _ in plan
                                if which != "v" for h in range(nheads)]
                    slabq = [slab_for(*slab_src[0]), slab_for(*slab_src[1])]
                    slab_i = 0
                    for which, nheads, wten, dst in plan:
                        for h in range(nheads):
                            if which == "v":
                                # v^T[d, tok] via long streams, then transpose
                                vt_ps = vtp.tile([128, SB], F32, name="vt_ps")
                                for c in range(HC):
                                    nc.tensor.matmul(
                                        vt_ps, wv_sb[:, c, h * 128:(h + 1) * 128],
                                        xt_t[:, c, :],
                                        start=(c == 0), stop=(c == HC - 1))
                                vt_sb = vt.tile([128, SB], BF, name="vt_sb")
                                nc.scalar.activation(
                                    out=vt_sb, in_=vt_ps,
                                    func=mybir.ActivationFunctionType.Copy)

                                def mk_trans(vt_sb=vt_sb, sb_i=sb_i, h=h):
                                    for t in range(SB // 128):
                                        tp_ps = tpp.tile([128, 128], BF,
                                                         name="tp_ps")
                                        nc.tensor.transpose(
                                            tp_ps,
                                            vt_sb[:, t * 128:(t + 1) * 128],
                                            id_sb)
                                        kt_i = sb_i * (SB // 128) + t
                                        nc.vector.tensor_copy(
                                            v_sb[:, kt_i, h * 128:(h + 1) * 128],
                                            tp_ps)
                                pending.append(mk_trans)
                                if len(pending) >= 2:
                                    pending.pop(0)()
                                continue
                            ps = pp.tile([128, SB], F32, name="ps_proj")
                            wslab = slabq[slab_i]
                            slab_i += 1
                            if slab_i + 1 < len(slab_src):
                                slabq.append(slab_for(*slab_src[slab_i + 1]))
                            for c in range(HC):
                                nc.tensor.matmul(ps, wslab[:, c, :], xt_t[:, c, :],
                                                 start=(c == 0), stop=(c == HC - 1))
                            # rope: dst = ps*cos + (R@ps)*sin_signed
                            qbf = rt.tile([128, SB], BF, name="rope_bf")
                            nc.scalar.activation(
                                out=qbf, in_=ps,
                                func=mybir.ActivationFunctionType.Copy)

                            def mk_rope(ps=ps, qbf=qbf, dst=dst, h=h, ssl=ssl):
                                rot = rp.tile([128, SB], F32, name="rot_ps")
                                nc.tensor.matmul(rot, r_sb, qbf,
                                                 start=True, stop=True)
                                t1 = rt.tile([128, SB], BF, name="rope_t1")
                                t2 = rt.tile([128, SB], BF, name="rope_t2")
                                nc.vector.tensor_mul(t1, ps, cos_sb[:, ssl])
                                nc.vector.tensor_mul(t2, rot, sin_sb[:, ssl])
                                nc.vector.tensor_add(dst[:, h, ssl], t1, t2)
                            pending.append(mk_rope)
                            if len(pending) >= 2:
                                pending.pop(0)()
                flush_pending()

            # ------------- phase 2+3: attention + output projection -------------
            with tc.tile_pool(name="wob", bufs=1) as wob, \
                 tc.tile_pool(name="aq", bufs=2) as aq, \
                 tc.tile_pool(name="ep", bufs=6) as ep, \
                 tc.tile_pool(name="es", bufs=3) as es, \
                 tc.tile_pool(name="rb", bufs=2) as rb, \
                 tc.tile_pool(name="ob", bufs=6) as ob, \
                 tc.tile_pool(name="sp", bufs=2, space="PSUM") as sp, \
                 tc.tile_pool(name="ap_", bufs=1, space="PSUM") as ap_, \
                 tc.tile_pool(name="op", bufs=1, space="PSUM") as op:
                wo_sb = wob.tile([128, NO, NQ, 512], BF, name="wo_sb")
                for n in range(NO):
                    nc.sync.dma_start(out=wo_sb[:, n], in_=wo[n])

                at_tiles = {}
                # O-projection emitted as fine-grained 4-matmul chunks injected
                # between attention k-tiles, so PE never idles in bursts (and
                # HAM never re-throttles). Single-n groups of 8 accumulating
                # matmuls, split in half, alternating between 2 PSUM tiles.
                oproj_chunks = []

                def emit_oproj_chunk():
                    qc, tt, n, half, g = oproj_chunks.pop(0)
                    at_qc = at_tiles[qc]
                    tsl = slice(tt * 128, (tt + 1) * 128)
                    name = "ps_o" + "ab"[g % 2]
                    if half == 0:
                        t = op.tile([128, 512], F32, name=name)
                        op_live[g % 2] = t
                        for c in range(4):
                            nc.tensor.matmul(
                                t, at_qc[:, c, tsl], wo_sb[:, n, c, :],
                                start=(c == 0), stop=False,
                                skip_group_check=True)
                    else:
                        t = op_live[g % 2]
                        for c in range(4, NQ):
                            nc.tensor.matmul(
                                t, at_qc[:, c, tsl], wo_sb[:, n, c, :],
                                start=False, stop=(c == NQ - 1),
                                skip_group_check=True)
                        o_t = ob.tile([128, 512], BF, name="o_t")
                        if n % 2 == 0:
                            nc.vector.tensor_copy(o_t, t)
                        else:
                            nc.scalar.activation(
                                out=o_t, in_=t,
                                func=mybir.ActivationFunctionType.Copy)
                        nc.sync.dma_start(
                            out=o[qc * QC + tt * 128:qc * QC + (tt + 1) * 128,
                                  n * 512:(n + 1) * 512],
                            in_=o_t)

                op_live = [None, None]

                def queue_oproj(qc):
                    for g in range(32):
                        n = g // 4          # n-major: early groups need only
                        tt = g % 4          # the first wo chunks
                        oproj_chunks.append((qc, tt, n, 0, g))
                        oproj_chunks.append((qc, tt, n, 1, g))

                norm_pending = []
                for qc in range(NQC):
                    qsl = slice(qc * QC, (qc + 1) * QC)
                    at_qc = aq.tile([128, NQ, QC], BF, name="at_qc")
                    at_tiles[qc] = at_qc
                    for hp in range(NHP):
                        if norm_pending:
                            norm_pending.pop(0)()
                        h0 = 2 * hp
                        kvh = h0 // (NQ // NKV)
                        attn2 = ap_.tile([128, 2, QC], F32, name="attn2")
                        esd = es.tile([128, 2, QC], BF, name="esd")

                        # scores+exp emitted one k-tile AHEAD of attn2 so the
                        # s2->exp->attn2 chain never serializes on PE's FIFO:
                        # exp(kc) runs on ACT while PE does attn2(kc-1)+oproj.
                        e2_tiles = {}

                        def emit_scores(kc):
                            s2 = sp.tile([128, 2, QC], F32, name="s2")
                            ksl = k_sb[:, kvh, kc * 128:(kc + 1) * 128]
                            nc.tensor.matmul(s2[:, 0, :], ksl, q_sb[:, h0, qsl],
                                             start=True, stop=True,
                                             skip_group_check=True)
                            nc.tensor.matmul(s2[:, 1, :], ksl,
                                             q_sb[:, h0 + 1, qsl],
                                             start=True, stop=True,
                                             skip_group_check=True)
                            e2 = esd if kc == 0 else ep.tile([128, 2, QC], BF,
                                                             name="e2")
                            nc.scalar.activation(
                                out=e2, in_=s2,
                                func=mybir.ActivationFunctionType.Exp,
                                scale=scale)
                            e2_tiles[kc] = e2

                        emit_scores(0)
                        for kc in range(KT):
                            if kc + 1 < KT:
                                emit_scores(kc + 1)
                            e2 = e2_tiles.pop(kc)
                            vsl = v_sb[:, kc, kvh * 128:(kvh + 1) * 128]
                            nc.tensor.matmul(attn2[:, 0, :], vsl, e2[:, 0, :],
                                             start=(kc == 0), stop=(kc == KT - 1),
                                             skip_group_check=True)
                            nc.tensor.matmul(attn2[:, 1, :], vsl, e2[:, 1, :],
                                             start=(kc == 0), stop=(kc == KT - 1),
                                             skip_group_check=True)
                            if kc > 0:
                                nc.vector.tensor_add(esd, esd, e2)
                            if oproj_chunks:
                                emit_oproj_chunk()
                        # den waits on the final esd DVE add -> give PE fill
                        for _ in range(2):
                            if oproj_chunks:
                                emit_oproj_chunk()
                        # evict raw attention on ACT (frees PSUM fast); the
                        # normalize chain is all-DVE and off the hot path
                        at_un = es.tile([128, 2, QC], BF, name="at_un")
                        nc.scalar.activation(
                            out=at_un, in_=attn2,
                            func=mybir.ActivationFunctionType.Copy)
                        # den via all-ones stationary: every output partition
                        # gets the full k-sum, so the reciprocal is already
                        # broadcast. Rides the s2 slot ring to stay in 8 banks.
                        den_bc = sp.tile([128, 2, QC], F32, name="s2")
                        nc.tensor.matmul(den_bc[:, 0, :], ones_sb, esd[:, 0, :],
                                         start=True, stop=True,
                                         skip_group_check=True)
                        nc.tensor.matmul(den_bc[:, 1, :], ones_sb, esd[:, 1, :],
                                         start=True, stop=True,
                                         skip_group_check=True)
                        rec2 = rb.tile([128, 2, QC], F32, name="rec2")
                        nc.vector.reciprocal_approx_fast(out=rec2, in_=den_bc)
                        norm_pending.append(
                            lambda at_qc=at_qc, h0=h0, at_un=at_un, rec2=rec2:
                            nc.vector.tensor_mul(at_qc[:, h0:h0 + 2, :], at_un,
                                                 rec2))
                    queue_oproj(qc)
                while norm_pending:
                    norm_pending.pop(0)()
                while oproj_chunks:
                    emit_oproj_chunk()
    nc.compile()
    return nc


def _rope_tables(position_ids_b, S_):
    """cos/sin tables in [d=128, s] layout, sin sign-folded for the half-swap."""
    pos = position_ids_b.astype(np.float32)
    inv_freq = (1.0 / (ROPE_THETA ** (np.arange(0, HEAD_DIM, 2, dtype=np.float32)
                                      / HEAD_DIM))).astype(np.float32)
    freqs = pos[:, None] * inv_freq[None, :]          # [s, 64]
    emb = np.concatenate([freqs, freqs], axis=1)      # [s, 128]
    cos = np.cos(emb).T.copy()                        # [128, s]
    sin = np.sin(emb).T.copy()
    sin[:64] *= -1.0                                  # sign-fold for swap rope
    return cos.astype(BF_NP), sin.astype(BF_NP)


def _prep_core_inputs(hidden_states, position_ids, Wq, Wk, Wv, Wo):
    rmat = np.zeros((128, 128), dtype=np.float32)
    for i in range(128):
        rmat[i, (i + 64) % 128] = 1.0
    rmat = rmat.astype(BF_NP)
    ident = np.eye(128, dtype=np.float32).astype(BF_NP)

    HC = HIDDEN // 128
    in_maps = []
    for t in range(TP):
        fq = slice(1024 * t, 1024 * (t + 1))
        fkv = slice(256 * t, 256 * (t + 1))
        # [h, p, c, m] layout: slab DMA reads 8KB contiguous per partition
        wq_t = np.ascontiguousarray(
            Wq[:, fq].reshape(HC, 128, 8, 128).transpose(2, 1, 0, 3)).astype(BF_NP)
        wk_t = np.ascontiguousarray(
            Wk[:, fkv].reshape(HC, 128, 2, 128).transpose(2, 1, 0, 3)).astype(BF_NP)
        wv_t = np.ascontiguousarray(Wv[:, fkv].reshape(HC, 128, 256)).astype(BF_NP)
        wo_t = np.ascontiguousarray(
            Wo[fq, :].reshape(8, 128, 8, 512).transpose(2, 1, 0, 3)).astype(BF_NP)
        for b in range(B):
            xt = np.ascontiguousarray(
                hidden_states[b].T.reshape(HC, 128, S)).astype(BF_NP)
            cos, sin = _rope_tables(position_ids[b], S)
            in_maps.append({"xt": xt, "wq": wq_t, "wk": wk_t, "wv": wv_t,
                            "wo": wo_t, "cos": cos, "sin": sin, "rmat": rmat,
                            "ident": ident})
    return in_maps


_NC_CACHE = {}


def kernel(hidden_states, position_ids, Wq, Wk, Wv, Wo):
    if "nc" not in _NC_CACHE:
        _NC_CACHE["nc"] = build_nc(FULL_CFG)
    nc = _NC_CACHE["nc"]
    in_maps = _prep_core_inputs(np.asarray(hidden_states), np.asarray(position_ids),
                                np.asarray(Wq), np.asarray(Wk),
                                np.asarray(Wv), np.asarray(Wo))
    res = run_bass_kernel_spmd(nc, in_maps, core_ids=list(range(8)))
    out = np.zeros((B, S, HIDDEN), dtype=np.float32)
    for t in range(TP):
        for b in range(B):
            out[b] += res.results[t * B + b]["o"].astype(np.float32)
    return out

